# revision 11
# baseline (speedup 1.0000x reference)
"""Trainium2 Bass kernel for nn_PhysicsLoss.

loss = mean|pred - target|
     + 0.1 * mean_b|soft_argmax_win(pred) - soft_argmax_win(target)| / T

where the window is [c-20, c+20) around c = argmax|target| per row, and
soft_argmax uses softmax(25*|x|) restricted to the window.

Sharding: pure data parallel, 8 cores x 512 rows. Each core streams its
[512, 6000] shard once (memory-bound) in uneven chunks (2600/2600/800 -- the
small last chunk shortens the serial tail). Chunk DMAs are parity-interleaved
across the two HWDGE rings (sync + scalar) so both rings carry identical byte
loads of mixed pred/target. Engine balance: DVE does subtract + block-max
reductions (bf16 2x-mode for tiles 0-2) + the top-8 argmax ops, ACT does
|.|+row-sum accumulation, |t|->bf16 for the fast block-max, the (i-c+0.5)^2
window mask and the softmax exps, GpSimd does the gather-offset math (tiles
0-2) and the indirect window gathers. Picks are kept window-local (offset
cancels in |pick_pred - pick_true|) and shipped as unnormalized (num, den)
pairs; the host does the divisions and the final reduction in f64.
"""

import sys

if "/opt/trn_rl_repo" not in sys.path:
    sys.path.insert(0, "/opt/trn_rl_repo")

from contextlib import ExitStack

import numpy as np

import concourse.bass as bass
import concourse.tile as tile
from concourse import bacc, mybir
from concourse._compat import with_exitstack
from concourse.bass_utils import run_bass_kernel_spmd

B, T = 4096, 6000
N_CORES = 8
ROWS = B // N_CORES      # 512 rows per core
P = 128                  # partitions
N_TILES = ROWS // P      # 4 row-tiles per core
BLK = 40                 # block width; window [c-20, c+20) spans <= 3 blocks
BLOCKS = T // BLK        # 150
WIN = 3 * BLK            # 120-wide gathered window
PICK_WIN = 20
BETA = 25.0
W_PICK = 0.1

CH_W = (2600, 2600, 800)     # chunk widths (all multiples of BLK)
CH_S = (0, 2600, 5200)       # chunk col starts
CH_B = (0, 65, 130)          # chunk starts in block space
N_CHUNKS = 3
CW = CH_W[0]                 # max chunk width (pool tile size)
N_BF16_TILES = 3             # tiles 0..2 use bf16 block-max; tile 3 f32

# consts table column layout
C_IOTA120 = 0            # [0,120): iota over window (local positions)
C_RB = WIN               # [120,124): col t = (t*128 + p) * 6000 (row base)
C_TOT = WIN + N_TILES    # 124

# per-tile output columns: [s1c0, s1c1, s1c2, wp, sp, wt, st]
OC = 7
OUT_COLS = OC * N_TILES  # 28

F32 = mybir.dt.float32
BF16 = mybir.dt.bfloat16
U32 = mybir.dt.uint32
I32 = mybir.dt.int32
ALU = mybir.AluOpType
ACTF = mybir.ActivationFunctionType
AXX = mybir.AxisListType.X


def _build_consts() -> np.ndarray:
    c = np.zeros((P, C_TOT), np.float32)
    c[:, C_IOTA120:C_IOTA120 + WIN] = np.arange(WIN)[None, :]
    for t in range(N_TILES):
        c[:, C_RB + t] = (t * P + np.arange(P)) * T
    return c


@with_exitstack
def _phys_loss_kernel(ctx: ExitStack, tc: tile.TileContext,
                      pred: bass.AP, target: bass.AP,
                      consts: bass.AP, out: bass.AP):
    nc = tc.nc

    cpool = ctx.enter_context(tc.tile_pool(name="cpool", bufs=1))
    ppool = ctx.enter_context(tc.tile_pool(name="ppool", bufs=6))
    tpool = ctx.enter_context(tc.tile_pool(name="tpool", bufs=6))
    dpool = ctx.enter_context(tc.tile_pool(name="dpool", bufs=2))
    spool = ctx.enter_context(tc.tile_pool(name="spool", bufs=2))

    ct = cpool.tile([P, C_TOT], F32)
    nc.gpsimd.dma_start(ct[:], consts[:, :])
    iota120 = ct[:, C_IOTA120:C_IOTA120 + WIN]

    pcs, tcs = {}, {}

    def issue_tile_dma(t):
        r0 = t * P
        for ci in range(N_CHUNKS):
            c0, w = CH_S[ci], CH_W[ci]
            g = t * N_CHUNKS + ci
            pc = ppool.tile([P, CW], F32, tag="pc")
            tcn = tpool.tile([P, CW], F32, tag="tc")
            # target first within each ring (pick chain gates the tail)
            if g % 2 == 0:
                nc.scalar.dma_start(tcn[:, :w], target[r0:r0 + P, c0:c0 + w])
                nc.sync.dma_start(pc[:, :w], pred[r0:r0 + P, c0:c0 + w])
            else:
                nc.sync.dma_start(tcn[:, :w], target[r0:r0 + P, c0:c0 + w])
                nc.scalar.dma_start(pc[:, :w], pred[r0:r0 + P, c0:c0 + w])
            pcs[(t, ci)] = pc
            tcs[(t, ci)] = tcn

    issue_tile_dma(0)
    issue_tile_dma(1)

    for t in range(N_TILES):
        use_bf16 = t < N_BF16_TILES
        outsb = spool.tile([P, OC], F32, tag="outsb")
        if use_bf16:
            bmax = spool.tile([P, BLOCKS], BF16, tag="bmaxb")
        else:
            bmax = spool.tile([P, BLOCKS], F32, tag="bmaxf")

        for ci in range(N_CHUNKS):
            w = CH_W[ci]
            nb = w // BLK
            b0 = CH_B[ci]
            pc = pcs.pop((t, ci))
            tcn = tcs.pop((t, ci))
            if use_bf16 and ci < 2:
                # |t| -> bf16 on ACT, then 2x-mode plain max on DVE
                abt = dpool.tile([P, CW], BF16, tag="abt")
                nc.scalar.activation(out=abt[:, :w], in_=tcn[:, :w],
                                     func=ACTF.Abs)
                nc.vector.tensor_reduce(
                    out=bmax[:, b0:b0 + nb],
                    in_=abt[:, :w].rearrange("p (b w) -> p b w", w=BLK),
                    axis=AXX, op=ALU.max)
            else:
                # abs-max in f32 (converted on write for bf16 tiles)
                nc.vector.tensor_reduce(
                    out=bmax[:, b0:b0 + nb],
                    in_=tcn[:, :w].rearrange("p (b w) -> p b w", w=BLK),
                    axis=AXX, op=ALU.max, apply_absolute_value=True)
            d = dpool.tile([P, CW], F32, tag="d")
            nc.vector.tensor_tensor(out=d[:, :w], in0=pc[:, :w],
                                    in1=tcn[:, :w], op=ALU.subtract)
            ad = dpool.tile([P, CW], BF16, tag="ad")
            nc.scalar.activation(out=ad[:, :w], in_=d[:, :w], func=ACTF.Abs,
                                 accum_out=outsb[:, ci:ci + 1])

        if t + 2 < N_TILES:
            issue_tile_dma(t + 2)

        # ---- phase 2: windowed soft-argmax picks ----
        # top-1 block index of |t| (bf16 comparisons are order-preserving)
        mx8 = spool.tile([P, 8], BF16 if use_bf16 else F32,
                         tag="mx8b" if use_bf16 else "mx8f")
        mi8 = spool.tile([P, 8], U32, tag="mi8")
        nc.vector.max(mx8[:], bmax[:])
        nc.vector.max_index(mi8[:], mx8[:], bmax[:])
        bstar = mi8[:, 0:1]

        # gather start gs40 = clamp(b*-1, 0, 147) * 40, flat DRAM offsets
        eng = nc.gpsimd if t < N_TILES - 1 else nc.vector
        g0 = spool.tile([P, 1], F32, tag="g0")
        eng.tensor_scalar(out=g0[:], in0=bstar, scalar1=1.0,
                          scalar2=0.0, op0=ALU.subtract, op1=ALU.max)
        gs40 = spool.tile([P, 1], F32, tag="gs40")
        eng.tensor_scalar(out=gs40[:], in0=g0[:],
                          scalar1=float(BLOCKS - 3), scalar2=float(BLK),
                          op0=ALU.min, op1=ALU.mult)
        offs_f = spool.tile([P, 1], F32, tag="offs_f")
        eng.tensor_scalar(out=offs_f[:], in0=ct[:, C_RB + t:C_RB + t + 1],
                          scalar1=gs40[:], op0=ALU.add, scalar2=None)
        offs_i = spool.tile([P, 1], I32, tag="offs_i")
        eng.tensor_copy(out=offs_i[:], in_=offs_f[:])

        tw = spool.tile([P, WIN], F32, tag="tw")
        nc.gpsimd.indirect_dma_start(
            out=tw[:], out_offset=None, in_=target[:, :],
            in_offset=bass.IndirectOffsetOnAxis(ap=offs_i[:], axis=1))
        pw = spool.tile([P, WIN], F32, tag="pw")
        nc.gpsimd.indirect_dma_start(
            out=pw[:], out_offset=None, in_=pred[:, :],
            in_offset=bass.IndirectOffsetOnAxis(ap=offs_i[:], axis=1))

        atw = spool.tile([P, WIN], F32, tag="atw")
        nc.scalar.activation(out=atw[:], in_=tw[:], func=ACTF.Abs)
        apw = spool.tile([P, WIN], F32, tag="apw")
        nc.scalar.activation(out=apw[:], in_=pw[:], func=ACTF.Abs)

        # exact (first) argmax position within the window, local coords
        w8 = spool.tile([P, 8], F32, tag="w8")
        cl8 = spool.tile([P, 8], U32, tag="cl8")
        nc.vector.max(w8[:], atw[:])
        nc.vector.max_index(cl8[:], w8[:], atw[:])
        cl = cl8[:, 0:1]

        # mask [cl-20, cl+20) via (i - cl + 0.5)^2 < 400
        negcl = spool.tile([P, 1], F32, tag="negcl")
        nc.vector.tensor_scalar(out=negcl[:], in0=cl, scalar1=-1.0,
                                scalar2=0.5, op0=ALU.mult, op1=ALU.add)
        sq = spool.tile([P, WIN], F32, tag="sq")
        nc.scalar.activation(out=sq[:], in_=iota120, func=ACTF.Square,
                             bias=negcl[:], scale=1.0)
        amt = spool.tile([P, WIN], F32, tag="amt")
        nc.vector.scalar_tensor_tensor(out=amt[:], in0=sq[:], scalar=400.0,
                                       in1=atw[:], op0=ALU.is_lt, op1=ALU.mult)
        amp = spool.tile([P, WIN], F32, tag="amp")
        nc.vector.scalar_tensor_tensor(out=amp[:], in0=sq[:], scalar=400.0,
                                       in1=apw[:], op0=ALU.is_lt, op1=ALU.mult)

        # target softmax: masked max is m = w8[0] exactly (argmax in mask);
        # masked-out exp(-25m) underflows to ~0, so no re-mask needed.
        negm = spool.tile([P, 1], F32, tag="negm")
        nc.vector.tensor_scalar(out=negm[:], in0=w8[:, 0:1], scalar1=-BETA,
                                op0=ALU.mult, scalar2=None)
        # pred needs its own masked max for exp stability
        mp = spool.tile([P, 1], F32, tag="mp")
        nc.vector.tensor_reduce(out=mp[:], in_=amp[:], axis=AXX, op=ALU.max)
        negmp = spool.tile([P, 1], F32, tag="negmp")
        nc.vector.tensor_scalar(out=negmp[:], in0=mp[:], scalar1=-BETA,
                                op0=ALU.mult, scalar2=None)

        et = spool.tile([P, WIN], F32, tag="et")
        nc.scalar.activation(out=et[:], in_=amt[:], func=ACTF.Exp,
                             scale=BETA, bias=negm[:],
                             accum_out=outsb[:, 6:7])          # st
        ep = spool.tile([P, WIN], F32, tag="ep")
        nc.scalar.activation(out=ep[:], in_=amp[:], func=ACTF.Exp,
                             scale=BETA, bias=negmp[:],
                             accum_out=outsb[:, 4:5])          # sp
        wdt = spool.tile([P, WIN], F32, tag="wdt")
        nc.vector.scalar_tensor_tensor(out=wdt[:], in0=et[:], scalar=1.0,
                                       in1=iota120, op0=ALU.mult, op1=ALU.mult,
                                       accum_out=outsb[:, 5:6])  # wt
        wdp = spool.tile([P, WIN], F32, tag="wdp")
        nc.vector.scalar_tensor_tensor(out=wdp[:], in0=ep[:], scalar=1.0,
                                       in1=iota120, op0=ALU.mult, op1=ALU.mult,
                                       accum_out=outsb[:, 3:4])  # wp

        nc.sync.dma_start(out[:, OC * t:OC * (t + 1)], outsb[:])


_COMPILED = None


def _get_compiled():
    global _COMPILED
    if _COMPILED is None:
        nc = bacc.Bacc("TRN2", target_bir_lowering=False, debug=False)
        pred = nc.dram_tensor("pred", [ROWS, T], F32, kind="ExternalInput").ap()
        target = nc.dram_tensor("target", [ROWS, T], F32, kind="ExternalInput").ap()
        consts = nc.dram_tensor("consts", [P, C_TOT], F32, kind="ExternalInput").ap()
        out = nc.dram_tensor("out", [P, OUT_COLS], F32, kind="ExternalOutput").ap()
        with tile.TileContext(nc) as tc:
            _phys_loss_kernel(tc, pred, target, consts, out)
        nc.compile()
        _COMPILED = nc
    return _COMPILED


def _run(pred: np.ndarray, target: np.ndarray, trace: bool = False):
    nc = _get_compiled()
    consts = _build_consts()
    pred = np.ascontiguousarray(pred, dtype=np.float32)
    target = np.ascontiguousarray(target, dtype=np.float32)
    in_maps = [
        {
            "pred": pred[k * ROWS:(k + 1) * ROWS],
            "target": target[k * ROWS:(k + 1) * ROWS],
            "consts": consts,
        }
        for k in range(N_CORES)
    ]
    res = run_bass_kernel_spmd(nc, in_maps, list(range(N_CORES)), trace=trace)
    s1 = 0.0
    perr = 0.0
    for k in range(N_CORES):
        o = res.results[k]["out"].astype(np.float64)
        for t in range(N_TILES):
            c = o[:, OC * t:OC * (t + 1)]
            s1 += c[:, 0:3].sum()
            pp = c[:, 3] / c[:, 4]
            pt = c[:, 5] / c[:, 6]
            perr += np.abs(pp - pt).sum()
    loss = s1 / (B * T)
    pick = (perr / B) / T
    total = np.array(loss + W_PICK * pick, dtype=np.float32)
    return total, res


def kernel(pred: np.ndarray, target: np.ndarray) -> np.ndarray:
    total, _ = _run(pred, target, trace=False)
    return total


# revision 13
# speedup vs baseline: 1.1494x; 1.1494x over previous
"""Trainium2 Bass kernel for nn_PhysicsLoss.

loss = mean|pred - target|
     + 0.1 * mean_b|soft_argmax_win(pred) - soft_argmax_win(target)| / T

where the window is [c-20, c+20) around c = argmax|target| per row, and
soft_argmax uses softmax(25*|x|) restricted to the window.

Sharding: pure data parallel, 8 cores x 512 rows. Each core streams its
[512, 6000] shard once (memory-bound) in uneven chunks; tile 0 leads with a
small 800-col chunk so compute starts early, tile 3 trails with one so the
final pick chain is short. Chunk DMAs are parity-interleaved across the two
HWDGE rings (sync + scalar), each ring carrying identical byte loads of mixed
pred/target. The sync engine's stream is pure DMA issues (all queued up
front) so ring feed never blocks behind compute dependencies; the scalar
ring's issues are staged two tiles ahead. DVE does subtract + blocked
abs-max + argmax top-8 ops + gather-offset math, ACT does |.|+row-sum
accumulation, the (i-c+0.5)^2 window mask and softmax exps, GpSimd does the
indirect window gathers and the final result DMAs. Picks stay window-local
(offset cancels in |pick_pred - pick_true|) and are shipped as unnormalized
(num, den) pairs split into engine-private output tensors; the host divides
and reduces in f64.
"""

import sys

if "/opt/trn_rl_repo" not in sys.path:
    sys.path.insert(0, "/opt/trn_rl_repo")

from contextlib import ExitStack

import numpy as np

import concourse.bass as bass
import concourse.tile as tile
from concourse import bacc, mybir
from concourse._compat import with_exitstack
from concourse.bass_utils import run_bass_kernel_spmd

B, T = 4096, 6000
N_CORES = 8
ROWS = B // N_CORES      # 512 rows per core
P = 128                  # partitions
N_TILES = ROWS // P      # 4 row-tiles per core
BLK = 40                 # block width; window [c-20, c+20) spans <= 3 blocks
BLOCKS = T // BLK        # 150
WIN = 3 * BLK            # 120-wide gathered window
PICK_WIN = 20
BETA = 25.0
W_PICK = 0.1

N_CHUNKS = 3
# per-tile chunk column widths / starts / block starts
CH_W = ((800, 2600, 2600),) + (((2600, 2600, 800),) * 3)
CH_S = ((0, 800, 3400),) + (((0, 2600, 5200),) * 3)
CH_B = ((0, 20, 85),) + (((0, 65, 130),) * 3)
CW = 2600                # max chunk width (pool tile size)

# consts table column layout
C_IOTA120 = 0            # [0,120): iota over window (local positions)
C_RB = WIN               # [120,124): col t = (t*128 + p) * 6000 (row base)
C_TOT = WIN + N_TILES    # 124

# ACT-written output: per tile [s1c0, s1c1, s1c2, st, sp]
OCA = 5
# DVE-written output: per tile [wt, wp]
OCB = 2

F32 = mybir.dt.float32
U32 = mybir.dt.uint32
I32 = mybir.dt.int32
ALU = mybir.AluOpType
ACTF = mybir.ActivationFunctionType
AXX = mybir.AxisListType.X


def _build_consts() -> np.ndarray:
    c = np.zeros((P, C_TOT), np.float32)
    c[:, C_IOTA120:C_IOTA120 + WIN] = np.arange(WIN)[None, :]
    for t in range(N_TILES):
        c[:, C_RB + t] = (t * P + np.arange(P)) * T
    return c


@with_exitstack
def _phys_loss_kernel(ctx: ExitStack, tc: tile.TileContext,
                      pred: bass.AP, target: bass.AP, consts: bass.AP,
                      out_a: bass.AP, out_b: bass.AP):
    nc = tc.nc

    cpool = ctx.enter_context(tc.tile_pool(name="cpool", bufs=1))
    ppool = ctx.enter_context(tc.tile_pool(name="ppool", bufs=6))
    tpool = ctx.enter_context(tc.tile_pool(name="tpool", bufs=6))
    dpool = ctx.enter_context(tc.tile_pool(name="dpool", bufs=2))
    spool = ctx.enter_context(tc.tile_pool(name="spool", bufs=2))

    ct = cpool.tile([P, C_TOT], F32)
    nc.gpsimd.dma_start(ct[:], consts[:, :])
    iota120 = ct[:, C_IOTA120:C_IOTA120 + WIN]

    outs_a = cpool.tile([P, OCA * N_TILES], F32, tag="outs_a")
    outs_b = cpool.tile([P, OCB * N_TILES], F32, tag="outs_b")

    # pre-allocate all chunk tiles (pool rotation = allocation order)
    pcs = {}
    tcs = {}
    for t in range(N_TILES):
        for ci in range(N_CHUNKS):
            pcs[(t, ci)] = ppool.tile([P, CW], F32, tag="pc",
                                      name=f"pc_{t}_{ci}")
            tcs[(t, ci)] = tpool.tile([P, CW], F32, tag="tc",
                                      name=f"tc_{t}_{ci}")

    def issue(t, ci, ring):
        r0 = t * P
        c0, w = CH_S[t][ci], CH_W[t][ci]
        g = t * N_CHUNKS + ci
        # parity: even g -> target on scalar ring, pred on sync ring
        tcn_eng = nc.scalar if g % 2 == 0 else nc.sync
        pc_eng = nc.sync if g % 2 == 0 else nc.scalar
        if ring == "sync":
            eng, dst = (tcn_eng, tcs) if tcn_eng is nc.sync else (pc_eng, pcs)
        else:
            eng, dst = (tcn_eng, tcs) if tcn_eng is nc.scalar else (pc_eng, pcs)
        src = target if dst is tcs else pred
        eng.dma_start(dst[(t, ci)][:, :w], src[r0:r0 + P, c0:c0 + w])

    def issue_both(t, ci):
        r0 = t * P
        c0, w = CH_S[t][ci], CH_W[t][ci]
        g = t * N_CHUNKS + ci
        if g % 2 == 0:
            nc.scalar.dma_start(tcs[(t, ci)][:, :w],
                                target[r0:r0 + P, c0:c0 + w])
            nc.sync.dma_start(pcs[(t, ci)][:, :w],
                              pred[r0:r0 + P, c0:c0 + w])
        else:
            nc.sync.dma_start(tcs[(t, ci)][:, :w],
                              target[r0:r0 + P, c0:c0 + w])
            nc.scalar.dma_start(pcs[(t, ci)][:, :w],
                                pred[r0:r0 + P, c0:c0 + w])

    # tiles 0-1: both rings up front; tiles 2-3: sync ring up front only
    # (SP's stream is pure issues, so blocking on buffer reuse is harmless;
    # the scalar ring's later issues are staged below to not stall ACT).
    for t in (0, 1):
        for ci in range(N_CHUNKS):
            issue_both(t, ci)
    for t in (2, 3):
        for ci in range(N_CHUNKS):
            issue(t, ci, "sync")

    for t in range(N_TILES):
        bmax = spool.tile([P, BLOCKS], F32, tag="bmax")

        for ci in range(N_CHUNKS):
            w = CH_W[t][ci]
            nb = w // BLK
            b0 = CH_B[t][ci]
            pc = pcs.pop((t, ci))
            tcn = tcs.pop((t, ci))
            d = dpool.tile([P, CW], F32, tag="d")
            nc.vector.tensor_tensor(out=d[:, :w], in0=pc[:, :w],
                                    in1=tcn[:, :w], op=ALU.subtract)
            nc.vector.tensor_reduce(
                out=bmax[:, b0:b0 + nb],
                in_=tcn[:, :w].rearrange("p (b w) -> p b w", w=BLK),
                axis=AXX, op=ALU.max, apply_absolute_value=True)
            ad = dpool.tile([P, CW], F32, tag="ad")
            nc.scalar.activation(out=ad[:, :w], in_=d[:, :w], func=ACTF.Abs,
                                 accum_out=outs_a[:, OCA * t + ci:OCA * t + ci + 1])

        if t + 2 < N_TILES:
            for ci in range(N_CHUNKS):
                issue(t + 2, ci, "scalar")

        # ---- phase 2: windowed soft-argmax picks ----
        mx8 = spool.tile([P, 8], F32, tag="mx8")
        mi8 = spool.tile([P, 8], U32, tag="mi8")
        nc.vector.max(mx8[:], bmax[:])
        nc.vector.max_index(mi8[:], mx8[:], bmax[:])
        bstar = mi8[:, 0:1]

        # gather start gs40 = clamp(b*-1, 0, 147) * 40, flat DRAM offsets
        g0 = spool.tile([P, 1], F32, tag="g0")
        nc.vector.tensor_scalar(out=g0[:], in0=bstar, scalar1=1.0,
                                scalar2=0.0, op0=ALU.subtract, op1=ALU.max)
        gs40 = spool.tile([P, 1], F32, tag="gs40")
        nc.vector.tensor_scalar(out=gs40[:], in0=g0[:],
                                scalar1=float(BLOCKS - 3), scalar2=float(BLK),
                                op0=ALU.min, op1=ALU.mult)
        offs_f = spool.tile([P, 1], F32, tag="offs_f")
        nc.vector.tensor_scalar(out=offs_f[:], in0=ct[:, C_RB + t:C_RB + t + 1],
                                scalar1=gs40[:], op0=ALU.add, scalar2=None)
        offs_i = spool.tile([P, 1], I32, tag="offs_i")
        nc.vector.tensor_copy(out=offs_i[:], in_=offs_f[:])

        tw = spool.tile([P, WIN], F32, tag="tw")
        nc.gpsimd.indirect_dma_start(
            out=tw[:], out_offset=None, in_=target[:, :],
            in_offset=bass.IndirectOffsetOnAxis(ap=offs_i[:], axis=1))
        pw = spool.tile([P, WIN], F32, tag="pw")
        nc.gpsimd.indirect_dma_start(
            out=pw[:], out_offset=None, in_=pred[:, :],
            in_offset=bass.IndirectOffsetOnAxis(ap=offs_i[:], axis=1))

        atw = spool.tile([P, WIN], F32, tag="atw")
        nc.scalar.activation(out=atw[:], in_=tw[:], func=ACTF.Abs)
        apw = spool.tile([P, WIN], F32, tag="apw")
        nc.scalar.activation(out=apw[:], in_=pw[:], func=ACTF.Abs)

        # exact (first) argmax position within the window, local coords
        w8 = spool.tile([P, 8], F32, tag="w8")
        cl8 = spool.tile([P, 8], U32, tag="cl8")
        nc.vector.max(w8[:], atw[:])
        nc.vector.max_index(cl8[:], w8[:], atw[:])
        cl = cl8[:, 0:1]

        # mask [cl-20, cl+20) via (i - cl + 0.5)^2 < 400
        negcl = spool.tile([P, 1], F32, tag="negcl")
        nc.vector.tensor_scalar(out=negcl[:], in0=cl, scalar1=-1.0,
                                scalar2=0.5, op0=ALU.mult, op1=ALU.add)
        sq = spool.tile([P, WIN], F32, tag="sq")
        nc.scalar.activation(out=sq[:], in_=iota120, func=ACTF.Square,
                             bias=negcl[:], scale=1.0)
        amt = spool.tile([P, WIN], F32, tag="amt")
        nc.vector.scalar_tensor_tensor(out=amt[:], in0=sq[:], scalar=400.0,
                                       in1=atw[:], op0=ALU.is_lt, op1=ALU.mult)
        amp = spool.tile([P, WIN], F32, tag="amp")
        nc.vector.scalar_tensor_tensor(out=amp[:], in0=sq[:], scalar=400.0,
                                       in1=apw[:], op0=ALU.is_lt, op1=ALU.mult)

        # target softmax: masked max is m = w8[0] exactly (argmax in mask);
        # masked-out exp(-25m) underflows to ~0, so no re-mask needed.
        negm = spool.tile([P, 1], F32, tag="negm")
        nc.vector.tensor_scalar(out=negm[:], in0=w8[:, 0:1], scalar1=-BETA,
                                op0=ALU.mult, scalar2=None)
        # pred needs its own masked max for exp stability
        mp = spool.tile([P, 1], F32, tag="mp")
        nc.vector.tensor_reduce(out=mp[:], in_=amp[:], axis=AXX, op=ALU.max)
        negmp = spool.tile([P, 1], F32, tag="negmp")
        nc.vector.tensor_scalar(out=negmp[:], in0=mp[:], scalar1=-BETA,
                                op0=ALU.mult, scalar2=None)

        et = spool.tile([P, WIN], F32, tag="et")
        nc.scalar.activation(out=et[:], in_=amt[:], func=ACTF.Exp,
                             scale=BETA, bias=negm[:],
                             accum_out=outs_a[:, OCA * t + 3:OCA * t + 4])   # st
        ep = spool.tile([P, WIN], F32, tag="ep")
        nc.scalar.activation(out=ep[:], in_=amp[:], func=ACTF.Exp,
                             scale=BETA, bias=negmp[:],
                             accum_out=outs_a[:, OCA * t + 4:OCA * t + 5])   # sp
        wdt = spool.tile([P, WIN], F32, tag="wdt")
        nc.vector.scalar_tensor_tensor(out=wdt[:], in0=et[:], scalar=1.0,
                                       in1=iota120, op0=ALU.mult, op1=ALU.mult,
                                       accum_out=outs_b[:, OCB * t:OCB * t + 1])  # wt
        wdp = spool.tile([P, WIN], F32, tag="wdp")
        nc.vector.scalar_tensor_tensor(out=wdp[:], in0=ep[:], scalar=1.0,
                                       in1=iota120, op0=ALU.mult, op1=ALU.mult,
                                       accum_out=outs_b[:, OCB * t + 1:OCB * t + 2])  # wp

    nc.gpsimd.dma_start(out_a[:, :], outs_a[:])
    nc.gpsimd.dma_start(out_b[:, :], outs_b[:])


_COMPILED = None


def _get_compiled():
    global _COMPILED
    if _COMPILED is None:
        nc = bacc.Bacc("TRN2", target_bir_lowering=False, debug=False)
        pred = nc.dram_tensor("pred", [ROWS, T], F32, kind="ExternalInput").ap()
        target = nc.dram_tensor("target", [ROWS, T], F32, kind="ExternalInput").ap()
        consts = nc.dram_tensor("consts", [P, C_TOT], F32, kind="ExternalInput").ap()
        out_a = nc.dram_tensor("out_a", [P, OCA * N_TILES], F32,
                               kind="ExternalOutput").ap()
        out_b = nc.dram_tensor("out_b", [P, OCB * N_TILES], F32,
                               kind="ExternalOutput").ap()
        with tile.TileContext(nc) as tc:
            _phys_loss_kernel(tc, pred, target, consts, out_a, out_b)
        nc.compile()
        _COMPILED = nc
    return _COMPILED


def _run(pred: np.ndarray, target: np.ndarray, trace: bool = False):
    nc = _get_compiled()
    consts = _build_consts()
    pred = np.ascontiguousarray(pred, dtype=np.float32)
    target = np.ascontiguousarray(target, dtype=np.float32)
    in_maps = [
        {
            "pred": pred[k * ROWS:(k + 1) * ROWS],
            "target": target[k * ROWS:(k + 1) * ROWS],
            "consts": consts,
        }
        for k in range(N_CORES)
    ]
    res = run_bass_kernel_spmd(nc, in_maps, list(range(N_CORES)), trace=trace)
    s1 = 0.0
    perr = 0.0
    for k in range(N_CORES):
        oa = res.results[k]["out_a"].astype(np.float64)
        ob = res.results[k]["out_b"].astype(np.float64)
        for t in range(N_TILES):
            s1 += oa[:, OCA * t:OCA * t + 3].sum()
            pt = ob[:, OCB * t] / oa[:, OCA * t + 3]
            pp = ob[:, OCB * t + 1] / oa[:, OCA * t + 4]
            perr += np.abs(pp - pt).sum()
    loss = s1 / (B * T)
    pick = (perr / B) / T
    total = np.array(loss + W_PICK * pick, dtype=np.float32)
    return total, res


def kernel(pred: np.ndarray, target: np.ndarray) -> np.ndarray:
    total, _ = _run(pred, target, trace=False)
    return total
